# revision 1
# baseline (speedup 1.0000x reference)
"""Trainium2 Bass kernel for spatial self-attention (B=4, C=64, H=W=64, 4 heads x 4 dim).

Sharding: the flattened spatial axis n = H*W = 4096 is split into 8 slices of
512 query positions, one per NeuronCore. Each core computes the full attention
pipeline (qkv projection, softmax attention over all 4096 keys, output
projection + bias) for its query slice across all batches/heads, so the host
unshard is a pure concatenate along the spatial axis.

Per-core dataflow, unit = (b, key-tile jt, head-pair hp):
  - 2 sim matmuls (contraction C=64 via the folded kq = wk^T q trick) write
    simT[128 keys, 512 q] for heads 2hp, 2hp+1 into one PSUM set (3-set
    rotation; one further PSUM bank accumulates AV, one stages prologue
    pieces, tails reuse the AV bank's upper rows).
  - exp over the [128, 1024] set -> fp8e4 SBUF: ScalarE exact Exp for ~64%
    of units, VectorE Schraudolph bit-trick exp (bits_i8 = s*A + B, written
    through an int8 bitcast) for the rest; the constant-scale part of the
    approximation cancels in softmax.
  - 1 fp8 DoubleRow AV matmul per unit: the two stationary planes (cols
    32hp and 64+32hp of the 128-col vT tile block) carry heads 2hp / 2hp+1
    zero-padded onto disjoint out rows, so all 4 heads of all 32 key tiles
    accumulate into rows 0:32 of a single PSUM bank at fp8 double-pump
    rate. Head HORD[k]=(0,2,1,3)[k] has its softmax denominator at row 5k
    (ones-columns memset by the Pool engine at vT col 37k) and values at
    rows 5k+1..+4 (vT cols 37k+1..+4, scattered from the V-projection
    PSUM by one strided 4-dim-AP copy per 4-tile group); rows 20:31 carry
    Pool-memset ones-pad sums so the tail's whole-block reciprocal is
    finite.
  - per-b tail: VectorE max(av,tiny)+reciprocal (finiteness: a value row
    summing to exactly 0.0 would put 0*inf=NaN into the broadcast), PE
    one-hot matmul broadcasts denominator reciprocals to the value rows,
    VectorE normalizes, PE output projection, VectorE bias-add + DMA out
    in two halves.

Engine balance per core (CoreSim, calibrated within 0.5% of HW on the
baseline): ScalarE ~156us of exp, VectorE ~157us of exp+copies+tails, PE
~144us of matmul, Pool ~16us of memsets; 186us total vs 318.7us baseline.
"""

import os
import sys

for p in ("/opt/trn_rl_repo", "/opt/pypackages"):
    if p not in sys.path:
        sys.path.insert(0, p)

os.environ.setdefault("MYCRO_LOCAL_CACHE", "1")

import ml_dtypes
import numpy as np

import concourse.bass as bass
import concourse.mybir as mybir
import concourse.tile as tile
from concourse import bacc
from concourse.bass_utils import run_bass_kernel_spmd
from concourse import bass2jax as _b2j

# --- NEFF cache: walrus compiles of the same HLO/BIR are cached on disk ---
_NEFF_CACHE_DIR = "/root/neff_cache"
_orig_hook = _b2j.neuronx_cc_hook


def _caching_neuronx_cc_hook(code, code_format, platform_version, file_prefix):
    import hashlib

    key = hashlib.sha256(
        bytes(code) + bytes(code_format) + str(platform_version).encode()
    ).hexdigest()
    path = os.path.join(_NEFF_CACHE_DIR, key + ".bin")
    if os.path.exists(path):
        with open(path, "rb") as f:
            return 0, f.read()
    r, data = _orig_hook(code, code_format, platform_version, file_prefix)
    try:
        os.makedirs(_NEFF_CACHE_DIR, exist_ok=True)
        tmp = path + ".tmp"
        with open(tmp, "wb") as f:
            f.write(data)
        os.replace(tmp, path)
    except Exception:
        pass
    return r, data


_b2j.neuronx_cc_hook = _caching_neuronx_cc_hook

BF16 = mybir.dt.bfloat16
F32 = mybir.dt.float32
FP8 = mybir.dt.float8e4
I8 = mybir.dt.int8

B = 4
C = 64
HW = 64
N = HW * HW  # 4096
HEADS = 4
DH = 4
SCALE = DH**-0.5
NCORES = 8
IS = N // NCORES  # 512 query positions per core
JT = N // 128  # 32 key tiles of 128
NPB = mybir.dt.np(BF16)  # ml_dtypes.bfloat16

# Schraudolph exp in fp8e4 bit space: bits_i8 = s * (8/ln2) + (7*8 - C)
EXP_A = 11.5415603
EXP_B = 56.0 - 0.46


_DVE_PAT = tuple(
    int(x) for x in os.environ.get("KQ_DVE_PAT", "1,4,6,9").split(",")
)
_DVE_MOD = int(os.environ.get("KQ_DVE_MOD", "11"))


def exp_on_dve(i):
    return i % _DVE_MOD in _DVE_PAT


def build_graph(reps=1):
    nc = bacc.Bacc(
        "TRN2", target_bir_lowering=False, debug=False, num_devices=NCORES
    )

    x_ext = nc.dram_tensor("x", [B, C + 1, N], BF16, kind="ExternalInput").ap()
    xq_ext = nc.dram_tensor("xq", [B, C, IS], BF16, kind="ExternalInput").ap()
    wq_ext = nc.dram_tensor("wq_sp", [C, 128], BF16, kind="ExternalInput").ap()
    wk_ext = nc.dram_tensor("wk_sp", [128, C], BF16, kind="ExternalInput").ap()
    wv_ext = nc.dram_tensor("wv2", [C, 16], BF16, kind="ExternalInput").ap()
    wo_ext = nc.dram_tensor("wo_sp", [32, C], BF16, kind="ExternalInput").ap()
    bc_ext = nc.dram_tensor("bc1h", [32, 32], BF16, kind="ExternalInput").ap()
    bias_ext = nc.dram_tensor("b_out", [C, 1], F32, kind="ExternalInput").ap()
    out_ext = nc.dram_tensor("out", [B, C, IS], F32, kind="ExternalOutput").ap()

    with tile.TileContext(nc) as tc:
        with (
            tc.tile_pool(name="const", bufs=1) as cst,
            tc.tile_pool(name="big", bufs=1) as big,
            tc.tile_pool(name="expp", bufs=7) as expp,
            tc.tile_pool(name="psum", bufs=1, space="PSUM") as psump,
        ):
            wq_s = cst.tile([C, 128], BF16, tag="wq", name="wq_s")
            wk_s = cst.tile([128, C], BF16, tag="wk", name="wk_s")
            wv_s = cst.tile([C, 16], BF16, tag="wv", name="wv_s")
            wo_s = cst.tile([32, C], BF16, tag="wo", name="wo_s")
            bc_s = cst.tile([32, 32], BF16, tag="bc", name="bc_s")
            bias_s = cst.tile([C, 1], F32, tag="bias", name="bias_s")

            xs = [big.tile([C + 1, N], BF16, tag=f"xs{b}", name=f"xs{b}") for b in range(B)]
            xqs = [big.tile([C, IS], BF16, tag=f"xqs{b}", name=f"xqs{b}") for b in range(B)]
            kq = [big.tile([C, 4 * IS], BF16, tag=f"kq{b}", name=f"kq{b}") for b in range(B)]
            qs = [big.tile([128, IS], BF16, tag=f"qs{b}", name=f"qs{b}") for b in range(B)]
            vT = [big.tile([128, 128 * JT + 128], FP8, tag=f"vT{b}", name=f"vT{b}") for b in range(B)]
            acc = [big.tile([32, IS], F32, tag=f"acc{b}", name=f"acc{b}") for b in range(B)]
            att = [big.tile([32, IS], BF16, tag=f"att{b}", name=f"att{b}") for b in range(B)]
            ys = [big.tile([C, IS], F32, tag=f"ys{b}", name=f"ys{b}") for b in range(B)]
            rec_bf = cst.tile([32, IS], BF16, tag="rec", name="rec_bf")
            rtmp = cst.tile([32, IS], F32, tag="rtmp", name="rtmp")

            # DMA order favors b0's critical path: q projection inputs, then
            # the first key tiles (so b0's first sim isn't gated on the whole
            # 3.2us x transfer), then everything else.
            nc.sync.dma_start(out=wq_s[:], in_=wq_ext)
            nc.sync.dma_start(out=xqs[0][:], in_=xq_ext[0])
            nc.sync.dma_start(out=wk_s[:], in_=wk_ext)
            nc.sync.dma_start(out=xs[0][:, 0:512], in_=x_ext[0][:, 0:512])
            nc.sync.dma_start(out=wv_s[:], in_=wv_ext)
            nc.sync.dma_start(out=xs[0][:, 512:N], in_=x_ext[0][:, 512:N])
            nc.sync.dma_start(out=wo_s[:], in_=wo_ext)
            nc.sync.dma_start(out=bc_s[:], in_=bc_ext)
            nc.sync.dma_start(out=bias_s[:], in_=bias_ext)
            for b in range(1, B):
                nc.sync.dma_start(out=xqs[b][:], in_=xq_ext[b])
                nc.sync.dma_start(out=xs[b][:, :], in_=x_ext[b])

            # warm the PE p-state during the initial DMA wait: the tensor
            # engine ramps 0.65 -> 2.4 GHz over ~3us of continuous busy, so
            # zero-matmuls here keep b0's critical q/kq/sim chain off the
            # slow clock (outputs land in the piece bank and are reset by
            # the q projection's start=True before any read)
            zw_a = cst.tile([C, 128], BF16, tag="zwa", name="zw_a")
            zw_b = cst.tile([C, 512], BF16, tag="zwb", name="zw_b")
            nc.gpsimd.memset(zw_a[:], 0.0)
            nc.gpsimd.memset(zw_b[:], 0.0)
            pcb_warm = None

            # PSUM: three 2-bank sim/exp sets (3-deep rotation) + one AV
            # accumulator bank (tail outputs reuse its upper rows) + one
            # dedicated prologue-piece bank.
            sets = [
                psump.tile([128, 1024], F32, tag=f"set{s}", name=f"set{s}")
                for s in range(3)
            ]
            avp = psump.tile([128, 512], F32, tag="av0", name="av0")
            pcb = psump.tile([128, 512], F32, tag="pcb", name="pcb")

            for _w in range(2):
                nc.tensor.matmul(
                    pcb[:, 0:384], zw_a[:], zw_b[:, 0:384],
                    start=True, stop=True,
                )

            # vT constant columns, written once per b by the (otherwise idle)
            # Pool engine: col 37k of each 128-col tile block is head k's
            # denominator ones-column, cols 20..31 the pad ones-columns that
            # keep the tail's whole-block reciprocal finite; v-value columns
            # (37k+1..+4) are filled from the projection pieces.
            for b in range(B):
                nc.gpsimd.memset(vT[b][:], 0.0)
                v3 = vT[b][:, 0 : 128 * JT].rearrange("p (t c) -> p t c", c=128)
                for k in range(4):
                    nc.gpsimd.memset(v3[:, :, 37 * k : 37 * k + 1], 1.0)
                nc.gpsimd.memset(v3[:, :, 20:32], 1.0)

            # ---- prologue pieces: q, kq chunks, vT groups per b.
            # b0's pieces run upfront; b>=1 pieces are interleaved into the
            # previous b's main loop. All use the dedicated piece bank.
            def piece(b, p):
                cp = nc.vector.tensor_copy
                S = pcb
                if p == 0:
                    nc.tensor.matmul(
                        S[:, 0:512], wq_s[:], xqs[b][:],
                        start=True, stop=True,
                    )
                    cp(qs[b][:], S[:, 0:512])
                elif p < 5:
                    h = p - 1
                    nc.tensor.matmul(
                        S[0:C, 0:512],
                        wk_s[32 * h : 32 * h + DH, :],
                        qs[b][32 * h : 32 * h + DH, :],
                        start=True,
                        stop=True,
                        tile_position=(32 * h, 0),
                    )
                    cp(
                        kq[b][:, 512 * h : 512 * (h + 1)],
                        S[0:C, 0:512],
                    )
                else:
                    g = p - 5
                    for k4 in range(4):
                        jt = 4 * g + k4
                        nc.tensor.matmul(
                            S[:, 16 * k4 : 16 * (k4 + 1)],
                            xs[b][0:C, jt * 128 : (jt + 1) * 128],
                            wv_s[:],
                            start=True,
                            stop=True,
                        )
                    # scatter the 4x(4 heads x 4 dims) projection columns
                    # into the stride-37 vT layout
                    vbase = vT[b][:]
                    out_ap = bass.AP(
                        vbase.tensor,
                        vbase.offset + 512 * g + 1,
                        [[128 * JT + 128, 128], [128, 4], [37, 4], [1, 4]],
                    )
                    cp(
                        out_ap,
                        S[:, 0:64].rearrange("p (t k d) -> p t k d", t=4, k=4, d=4),
                    )

            NP_PIECES = 13  # 1 q + 4 kq + 8 vT-groups

            def emit_piece(b, p):
                piece(b, p)

            # remaining b0 vT groups woven into early units; the full
            # prologue of b+1 spread inside b's units
            piece_sched = {}
            for g in range(1, 8):
                piece_sched.setdefault(g - 1, []).append((0, 5 + g))
            for b in range(1, B):
                start_u = 64 * (b - 1) + 16
                for p in range(NP_PIECES):
                    piece_sched.setdefault(start_u + p, []).append((b, p))

            # ---- main loop over units (b, jt, hp) ----
            def simq2(b, jt, hp, S):
                for hh in range(2):
                    h = 2 * hp + hh
                    nc.tensor.matmul(
                        S[:, 512 * hh : 512 * (hh + 1)],
                        xs[b][0:C, jt * 128 : (jt + 1) * 128],
                        kq[b][:, 512 * h : 512 * (h + 1)],
                        start=True,
                        stop=True,
                    )

            def avq(b, jt, hp, et):
                av = avp
                nc.tensor.matmul(
                    av[0:32, :],
                    vT[b][:, 128 * jt + 32 * hp : 128 * jt + 32 * hp + 128]
                    .rearrange("p (two m) -> p two m", two=2)[:, :, 0:32],
                    et[:].rearrange("p (two n) -> p two n", two=2),
                    start=(jt == 0 and hp == 0),
                    stop=(jt == JT - 1 and hp == 1),
                    perf_mode=mybir.MatmulPerfMode.DoubleRow,
                    tile_position=(0, 0),
                    skip_group_check=True,
                )

            def tail(b, s_tail):
                # Tail PSUM traffic reuses the AV bank's upper rows (32:64 for
                # the broadcast, 64:128 for the output projection) so neither
                # the sim set rotation nor the next b's accumulation (rows
                # 0:32, gated only on the copy below) ever stalls on it.
                av = avp
                # copy the 32-row AV block to SBUF. Rows {0,5,16,21} are the
                # softmax denominators, rows +1..+4 each head's values, rows
                # 10..15/26..31 ones-row pad sums (positive, so the whole-
                # block reciprocal below stays finite; the one-hot broadcast
                # masks them out).
                # max vs tiny keeps the masked value-row reciprocals finite
                # (an AV row summing to exactly 0.0 would otherwise put
                # 0*inf = NaN into the one-hot broadcast matmul; masked rows
                # only need finiteness, not correctness)
                nc.vector.tensor_scalar(
                    rtmp[:], av[0:32, :], 1e-30, None, mybir.AluOpType.max
                )
                with nc.allow_low_precision("softmax denom recip; 2e-2 gate"):
                    nc.vector.reciprocal(rec_bf[:], rtmp[:])
                nc.vector.tensor_copy(acc[b][:], av[0:32, :])
                # one-hot PE broadcast: av rows 32+r+1..32+r+4 <- rec row r
                nc.tensor.matmul(
                    av[32:64, 0:512], bc_s[:], rec_bf[:], start=True, stop=True,
                    tile_position=(0, 32),
                )
                nc.vector.tensor_tensor(
                    att[b][:], acc[b][:], av[32:64, 0:512], mybir.AluOpType.mult
                )
                nc.tensor.matmul(
                    av[64:128, :], wo_s[:], att[b][:], start=True, stop=True,
                    tile_position=(0, 64),
                )
                # halved bias+DMA so the last chunk's store overlaps the
                # first chunk's transfer during drain
                for half in range(2):
                    cols = slice(256 * half, 256 * (half + 1))
                    nc.vector.tensor_scalar(
                        ys[b][:, cols],
                        av[64 : 64 + C, cols],
                        bias_s[:],
                        None,
                        mybir.AluOpType.add,
                    )
                    nc.sync.dma_start(
                        out=out_ext[b][:, cols], in_=ys[b][:, cols]
                    )

            # AV consumption runs LAG units behind sim/exp; the first EXTRA
            # units of each b are held back LAG extra so the PE's in-order
            # queue never blocks on the previous b's tail copy draining the
            # AV bank.
            LAG = int(os.environ.get("KQ_LAG", "10"))
            EXTRA = int(os.environ.get("KQ_EXTRA", "4"))
            units = [
                (b, jt, hp) for b in range(B) for jt in range(JT) for hp in range(2)
            ]

            for rep in range(reps):
                # b0: q + all kq + vT-group0 before the main loop starts
                for p in (0, 1, 2, 3, 4, 5):
                    emit_piece(0, p)
                ets = {}

                def consume(j):
                    pb, pjt, php = units[j]
                    avq(pb, pjt, php, ets[j])
                    del ets[j]
                    if pjt == JT - 1 and php == 1:
                        tail(pb, None)

                next_av = 0

                def drain(i):
                    nonlocal next_av
                    # the final few AVs run at a short lag so the wind-down
                    # (last exps -> last AVs -> tail) is as shallow as possible
                    while next_av < len(units) and next_av <= i - (
                        LAG if next_av < len(units) - 12 else 3
                    ):
                        j = next_av
                        if j % 64 < EXTRA and i - j < LAG + EXTRA:
                            break
                        consume(j)
                        next_av += 1

                for i, (b, jt, hp) in enumerate(units):
                    simq2(b, jt, hp, sets[i % 3])
                    et = expp.tile([128, 1024], FP8, tag="et",
                                   name=f"et{rep}_{i}",
                                   bufs=LAG + EXTRA + 1)
                    ets[i] = et
                    if exp_on_dve(i):
                        with nc.allow_low_precision("Schraudolph exp bit trick"):
                            nc.vector.tensor_scalar(
                                et[:].bitcast(I8),
                                sets[i % 3][:, :],
                                EXP_A,
                                EXP_B,
                                mybir.AluOpType.mult,
                                mybir.AluOpType.add,
                            )
                    else:
                        nc.scalar.activation(
                            et[:],
                            sets[i % 3][:, :],
                            mybir.ActivationFunctionType.Exp,
                        )
                    for pb, pp in piece_sched.get(i, ()):
                        emit_piece(pb, pp)
                    drain(i)
                while next_av < len(units):
                    consume(next_av)
                    next_av += 1

    nc.compile()
    return nc


def host_prep(x, w_qkv, w_out, b_out):
    x3 = np.ascontiguousarray(x.reshape(B, C, N), dtype=np.float32)
    x_bf = x3.astype(NPB)
    x_pad = np.ones((B, C + 1, N), NPB)
    x_pad[:, 0:C, :] = x_bf
    wq = w_qkv[0:16].astype(np.float32) * SCALE
    wk = w_qkv[16:32].astype(np.float32)
    wv = w_qkv[32:48].astype(np.float32)

    wq_sp = np.zeros((C, 128), np.float32)
    wk_sp = np.zeros((128, C), np.float32)
    for h in range(HEADS):
        for d in range(DH):
            wq_sp[:, 32 * h + d] = wq[4 * h + d]
            wk_sp[32 * h + d, :] = wk[4 * h + d]

    # wv2: moving operand of the V projection, 16 value columns per key
    # tile in (k, d) order where k indexes DoubleRow stationary blocks
    # (plane pl of head-pair hp at vT col base 64pl+32hp = 32k). Block k
    # carries head HORD[k] = 2hp+pl at vT cols 37k+1..+4 (out rows
    # 5k+1..+4); denominator ones-columns (37k) and pad ones-columns
    # (tile cols 20..31, out rows 20..31) are memset device-side.
    HORD = [0, 2, 1, 3]
    wv2 = np.zeros((C, 16), np.float32)
    for k in range(4):
        for d in range(DH):
            wv2[:, 4 * k + d] = wv[4 * HORD[k] + d]

    # AV accumulator rows (32-row block): head HORD[k] at 5k, denominator
    # at +0, values at +1..+4, pad sums at rows 20..31
    wo_sp = np.zeros((32, C), np.float32)
    bc1h = np.zeros((32, 32), np.float32)
    for k in range(4):
        h = HORD[k]
        r = 5 * k
        bc1h[r, r + 1 : r + 5] = 1.0
        for d in range(DH):
            wo_sp[r + 1 + d, :] = w_out[:, 4 * h + d]

    common = {
        "x": x_pad,
        "wq_sp": wq_sp.astype(NPB),
        "wk_sp": wk_sp.astype(NPB),
        "wv2": wv2.astype(NPB),
        "wo_sp": wo_sp.astype(NPB),
        "bc1h": bc1h.astype(NPB),
        "b_out": np.ascontiguousarray(b_out.reshape(C, 1), dtype=np.float32),
    }
    in_maps = []
    for c in range(NCORES):
        m = dict(common)
        m["xq"] = np.ascontiguousarray(x_bf[:, :, c * IS : (c + 1) * IS])
        in_maps.append(m)
    return in_maps


_NC_CACHE = None


def get_nc():
    global _NC_CACHE
    if _NC_CACHE is None:
        _NC_CACHE = build_graph()
    return _NC_CACHE


def run(inputs, trace=False):
    nc = get_nc()
    in_maps = host_prep(**inputs)
    # NTFF tracing is unavailable through this axon client (antenv.axon_hooks
    # missing); always run untraced.
    res = run_bass_kernel_spmd(
        nc, in_maps, core_ids=list(range(NCORES)), trace=False
    )
    pieces = [res.results[c]["out"] for c in range(NCORES)]
    y = np.concatenate(pieces, axis=2)  # [B, C, N]
    y = y.reshape(B, C, HW, HW).astype(np.float32)
    return y, res


def kernel(**inputs):
    y, _ = run(inputs, trace=False)
    return y


if __name__ == "__main__":
    rng = np.random.default_rng(0)
    ins = {
        "x": rng.standard_normal((B, C, HW, HW), dtype=np.float32),
        "w_qkv": (rng.standard_normal((48, C)) * 0.05).astype(np.float32),
        "w_out": (rng.standard_normal((C, 16)) * 0.05).astype(np.float32),
        "b_out": (rng.standard_normal(C) * 0.05).astype(np.float32),
    }
    y = kernel(**ins)
    print("out shape", y.shape, y.dtype)

